# revision 22
# baseline (speedup 1.0000x reference)
"""Multi-head attention (B=8, H=8, S=1024, d=128) on 8 TRN2 NeuronCores.

Strategy
--------
- 2D sharding over (batch, head): the 64 (batch, head) attention
  problems are dealt to the 8 cores so that every core gets the same
  mix of "large-mask" and "small-mask" batches (the number of 128-wide
  key tiles surviving seq_mask compaction varies per batch, and the
  scalar engine's exp throughput is the kernel bottleneck). Each core
  runs n_a head-slots with kt_a key tiles and n_b = 8-n_a slots with
  kt_b tiles; the (kt_a, n_a, kt_b) program shape is identical on all
  cores (SPMD), only the data differs.
- Host-side prep (layout only): per batch, compact keys/values to the
  seq_mask-selected rows (zero-padded to the segment's k-tile count),
  pre-transpose Q and compacted K so the contraction dim (d) lands on
  SBUF partitions, and cast matmul operands to fp16. V is augmented per
  head with a 129th "indicator" column (1 for real keys, 0 for padding)
  so the softmax denominator falls out of the AV matmul.
- Device math per head-slot:
    logitsT[k, q] = K_h^T.T @ Q_h^T          (PE, M=128 k-tiles, N=512)
    W^T[k, q]     = exp(logitsT * d^-0.5)    (ACT, PSUM -> SBUF fp16,
                                              batched in alternating
                                              1536/1024-col groups to
                                              amortize ACTIVATE overhead)
    out[q, 129]   = sum_kt W^T[kt,qtile].T @ [V_h[kt] | ind[kt]]
                                             (PE, M=128 q-tiles, N=129,
                                              PSUM accumulation over kt;
                                              col 128 = denominator)
    osb[q, d]     = out[:, :128] * recip(out[:, 128])  (DVE)
  The learned scalar bias b cancels in softmax (shift invariance) and
  the -1e30 masking is equivalent to dropping masked keys, which the
  compaction does exactly.
- Software pipelining: AV+epilogue of slot s-1 are interleaved into the
  QK group stream of slot s so the scalar engine never waits at slot
  boundaries. A short burst of dummy matmuls at kernel start warms the
  PE HAM clock gate while the first input DMAs are in flight.
- Output per head-slot is DMA'd as a contiguous [128, 1024] fp16 block
  ([q-within-tile, (q-tile, d)]); the host reassembles [S, D] and
  handles the degenerate all-masked batch (uniform average).
"""
from contextlib import ExitStack

import numpy as np

import concourse.bacc as bacc
import concourse.mybir as mybir
import concourse.tile as tile
from concourse.bass_utils import run_bass_kernel_spmd

F32 = mybir.dt.float32
F16 = mybir.dt.float16
Exp = mybir.ActivationFunctionType.Exp

B, S, D, H = 8, 1024, 1024, 8
DH = D // H              # 128, head dim = one partition tile
SCALE = float(DH) ** -0.5
NQT = S // 128           # 8 q-tiles per head

_NC_CACHE: dict[tuple, object] = {}

# build options (overridable for profiling experiments)
OPTS: dict = {}


def _plan_groups(n_kts):
    """Exp groups per slot with STRICT GLOBAL A/B pool alternation (the
    pool ping-pong carries across slot boundaries, so a slot may start
    on either pool). Slot 0 leads with a 512-col group so the first exp
    fires after a single QK matmul."""
    plans, a = [], True
    for s, cols in enumerate(n_kts):
        rem, pos, groups = cols, 0, []
        first = 512 if s == 0 else None
        while rem:
            size = min(1536 if a else 1024, rem)
            if first is not None:
                size, first = first, None
            groups.append((a, pos, size))
            pos += size
            rem -= size
            a = not a
        plans.append(groups)
    return plans


def _build(kt_a: int, kt_b: int, n_a: int, opts: dict | None = None):
    """Per-core kernel: n_a head-slots with kt_a k-tiles, then 8-n_a
    slots with kt_b k-tiles."""
    opts = opts or {}
    w_bufs = opts.get("w_bufs", 2)
    o_bufs = opts.get("o_bufs", 2)
    n_warm = opts.get("n_warm", 8)
    n_b = H - n_a
    KPA, KPB = kt_a * 128, kt_b * 128
    VWA, VWB = n_a * 129, n_b * 129
    nc = bacc.Bacc("TRN2", target_bir_lowering=False, debug=False)

    q_t = nc.dram_tensor("q_t", [D, S], F16, kind="ExternalInput")
    k_a = nc.dram_tensor("k_a", [n_a * DH, KPA], F16, kind="ExternalInput")
    k_b = nc.dram_tensor("k_b", [n_b * DH, KPB], F16, kind="ExternalInput")
    v_a = nc.dram_tensor("v_a", [KPA, VWA], F16, kind="ExternalInput")
    v_b = nc.dram_tensor("v_b", [KPB, VWB], F16, kind="ExternalInput")
    out_t = nc.dram_tensor("out_t", [H * 128, S], F16, kind="ExternalOutput")

    def po_off(qi):
        g, j = divmod(qi, 3)
        return g * 512 + j * 129

    with tile.TileContext(nc) as tc, ExitStack() as ctx:
        sb_k = ctx.enter_context(tc.tile_pool(name="sb_k", bufs=1))
        sb_q = ctx.enter_context(tc.tile_pool(name="sb_q", bufs=1))
        sb_v = ctx.enter_context(tc.tile_pool(name="sb_v", bufs=1))
        sb_wm = ctx.enter_context(tc.tile_pool(name="sb_wm", bufs=1))
        sb_w = ctx.enter_context(tc.tile_pool(name="sb_w", bufs=w_bufs))
        sb_o = ctx.enter_context(tc.tile_pool(name="sb_o", bufs=o_bufs))
        ps_a = ctx.enter_context(tc.tile_pool(name="ps_a", bufs=1, space="PSUM"))
        ps_b = ctx.enter_context(tc.tile_pool(name="ps_b", bufs=1, space="PSUM"))
        ps_o = ctx.enter_context(tc.tile_pool(name="ps_o", bufs=1, space="PSUM"))

        kalls = [sb_k.tile([128, n_a * KPA], F16, tag="ka", name="ka"),
                 sb_k.tile([128, n_b * KPB], F16, tag="kb", name="kb")]
        qall = sb_q.tile([128, H * S], F16)
        valls = [sb_v.tile([128, kt_a * VWA], F16, tag="va", name="va"),
                 sb_v.tile([128, kt_b * VWB], F16, tag="vb", name="vb")]
        kts = [k_a, k_b]
        vts = [v_a, v_b]
        KPs, VWs = (KPA, KPB), (VWA, VWB)

        # --- PE warmup: dense dummy matmuls while the first DMAs fly, so
        # the HAM clock gate reaches 8/8 before real work arrives.
        if n_warm:
            wl = sb_wm.tile([128, 128], F16)
            wr = sb_wm.tile([128, 512], F16)
            wo = sb_wm.tile([128, 1], F32)
            nc.gpsimd.memset(wl[:], 0.0)
            nc.gpsimd.memset(wr[:], 0.0)
            # Dummy first activation: hoists the auto-inserted ACT table
            # load to the head of the scalar queue so it runs at boot,
            # not after the first QK group is ready.
            nc.scalar.activation(wo[:], wl[:, 0:1], Exp)
            warm_po = ps_o.tile([128, 1536], F32, tag="po", name="po_warm")
            for _ in range(n_warm):
                nc.tensor.matmul(warm_po[:, 0:512], wl[:], wr[:],
                                 start=True, stop=True, skip_group_check=True)

        # --- Input DMAs: a single need-ordered piece list dealt
        # round-robin to the sync/gpsimd/vector queues (NOT scalar, so
        # the auto-inserted ACT table load runs at boot; NOT tensor, so
        # warmup/QK issue immediately). Slot-0 operands are split in
        # half so two queues fetch them concurrently.
        def k_piece(seg, si, c0, c1):
            KP = KPs[seg]
            return (kalls[seg][:, si * KP + c0:si * KP + c1],
                    kts[seg].ap()[si * DH:(si + 1) * DH, c0:c1])

        def q_piece(s, c0, c1):
            return (qall[:, s * S + c0:s * S + c1],
                    q_t.ap()[s * DH:(s + 1) * DH, c0:c1])

        def v_piece(seg, kt):
            VW = VWs[seg]
            return (valls[seg][:, kt * VW:(kt + 1) * VW],
                    vts[seg].ap()[kt * 128:(kt + 1) * 128, :])

        def seg_si(s):
            return (0, s) if s < n_a else (1, s - n_a)

        pieces = []
        after = {s: [] for s in range(H)}
        # v_a tiles land after slot 1/2's k+q, v_b after slot n_a(+1)'s.
        va = [v_piece(0, kt) for kt in range(kt_a)]
        vb = [v_piece(1, kt) for kt in range(kt_b)]
        after[1] += va[:2]
        after[min(2, H - 1)] += va[2:]
        after[min(n_a, H - 1)] += vb[:2]
        after[min(n_a + 1, H - 1)] += vb[2:]
        for s in range(H):
            seg, si = seg_si(s)
            KP = KPs[seg]
            if s == 0:
                pieces += [k_piece(seg, si, 0, 256), q_piece(s, 0, 512),
                           q_piece(s, 512, S), k_piece(seg, si, 256, KP)]
            else:
                pieces += [k_piece(seg, si, 0, KP), q_piece(s, 0, S)]
            pieces += after[s]
        # sync+gpsimd alternate the first pieces; the scalar queue takes
        # three mid-priority pieces (its issues start late, after the
        # dummy activation, and must not crowd the exp stream).
        qs = [nc.sync, nc.gpsimd]
        order = []
        for i in range(len(pieces)):
            order.append(qs[i % 2] if i < 6 or i >= 9 else nc.scalar)
        for eng, (dst, src) in zip(order, pieces):
            eng.dma_start(dst, src)

        def emit_qk_group(job, gi, ring, groups):
            s, q0, nq = job["s"], job["q0"], job["nq"]
            seg = 0 if s < n_a else 1
            si = s if s < n_a else s - n_a
            KP = KPs[seg]
            a, start, size = groups[gi]
            pool = ps_a if a else ps_b
            cap = 1536 if a else 1024
            pl = pool.tile([128, cap], F32, tag="pl" + ("A" if a else "B"),
                           name=f"pl_{s}_{q0}_{start}")
            for local in range(0, size, 512):
                gcol = start + local
                kt, qh = divmod(gcol, nq)
                lhsT = kalls[seg][:, si * KP + kt * 128: si * KP + (kt + 1) * 128]
                nc.tensor.matmul(
                    pl[:, local:local + 512],
                    lhsT, qall[:, s * S + q0 + qh:s * S + q0 + qh + 512],
                    start=True, stop=True)
            nc.scalar.activation(
                ring[:, start:start + size], pl[:, 0:size], Exp, scale=SCALE)

        def emit_av_kt(job, ring, kt, po):
            s, q0, nq, nqt = job["s"], job["q0"], job["nq"], job["nqt"]
            n_kt = job["kt"]
            seg = 0 if s < n_a else 1
            si = s if s < n_a else s - n_a
            VW = VWs[seg]
            first, last = kt == 0, kt == n_kt - 1
            rhs = valls[seg][:, kt * VW + si * 129: kt * VW + (si + 1) * 129]
            for qi in range(nqt):
                off = po_off(qi)
                # start=True clears the has_written bits of the WHOLE
                # bank, so only the first matmul touching each bank may
                # carry it; the other regions' first writes rely on
                # their (now cleared) bits selecting overwrite mode.
                nc.tensor.matmul(
                    po[:, off:off + 129],
                    ring[:, kt * nq + qi * 128: kt * nq + (qi + 1) * 128],
                    rhs, start=first and qi % 3 == 0, stop=last,
                    skip_group_check=True)

        def emit_epilogue(job, po, last=False):
            s, q0, nqt = job["s"], job["q0"], job["nqt"]
            oal = sb_o.tile([128, 1536], F16, tag="oal", name=f"oal_{s}_{q0}")
            rst = sb_o.tile([128, 9], F32, tag="rst", name=f"rst_{s}_{q0}")
            osb = sb_o.tile([128, S], F16, tag="osb", name=f"osb_{s}_{q0}")
            if not last:
                # One big copy releases the po banks fast (the next job's
                # AV matmuls head-of-line block the PE queue on it).
                hi = po_off(nqt - 1) + 129
                nc.vector.tensor_copy(oal[:, 0:hi], po[:, 0:hi])
            for g in range((nqt + 2) // 3):
                cnt = min(3, nqt - 3 * g)
                base = g * 512
                if last:
                    # po release doesn't matter anymore; fully pipeline
                    # copy -> recip -> muls -> store per bank-group.
                    nc.vector.tensor_copy(
                        oal[:, base:base + cnt * 129],
                        po[:, base:base + cnt * 129])
                nc.vector.reciprocal(
                    rst[:, g * 3:g * 3 + cnt],
                    oal[:, base + 128:base + cnt * 129:129])
                for j in range(cnt):
                    qi = g * 3 + j
                    nc.vector.tensor_scalar_mul(
                        osb[:, qi * 128:(qi + 1) * 128],
                        oal[:, g * 512 + j * 129:g * 512 + j * 129 + 128],
                        rst[:, qi:qi + 1])
                c0, c1 = g * 384, g * 384 + cnt * 128
                qs[(s + g) % len(qs)].dma_start(
                    out_t.ap()[s * 128:(s + 1) * 128, q0 + c0:q0 + c1],
                    osb[:, c0:c1])

        # Boundary-level software pipeline. Per job j the PE queue gets:
        #   QK(j, g0) | AV(j-1, kt 0..last-1) | QK(j, g1) | AV(j-1, last)
        #   | QK(j, g2..) | epilogue(j-1)
        # so exp(j, g0) starts the moment exp(j-1, last) finishes, and the
        # previous job's AV (whose exps are long done) fills PE idle time
        # without head-of-line blocking the QK stream. The final slot is
        # split into two query-half jobs so the last epilogue + store
        # overlap the second half's exp stream instead of trailing it.
        n_kts = [kt_a] * n_a + [kt_b] * n_b
        jobs = []
        for s in range(H):
            if s == H - 1:
                jobs.append({"s": s, "q0": 0, "nq": 512, "nqt": 4,
                             "kt": n_kts[s]})
                jobs.append({"s": s, "q0": 512, "nq": 512, "nqt": 4,
                             "kt": n_kts[s]})
            else:
                jobs.append({"s": s, "q0": 0, "nq": S, "nqt": NQT,
                             "kt": n_kts[s]})
        plans = _plan_groups([j["kt"] * j["nq"] for j in jobs])
        rings, pos = {}, {}
        nj = len(jobs)
        for ji, job in enumerate(jobs):
            grp = plans[ji]
            rings[ji] = sb_w.tile([128, kt_a * 1024], F16, tag="ring",
                                  name=f"ring_{ji}")
            if ji >= 1:
                pos[ji - 1] = ps_o.tile([128, 1536], F32, tag="po",
                                        name=f"po_{ji - 1}")
            final = ji == nj - 1
            for gi in range(len(grp)):
                emit_qk_group(job, gi, rings[ji], grp)
                if ji >= 1 and gi == 0:
                    # During the final job, finish the previous job's AV
                    # and epilogue as early as possible so they overlap
                    # the remaining exp groups instead of trailing them.
                    lim = jobs[ji - 1]["kt"] if final else \
                        jobs[ji - 1]["kt"] - 1
                    for kt in range(lim):
                        emit_av_kt(jobs[ji - 1], rings[ji - 1], kt,
                                   pos[ji - 1])
                    if final:
                        emit_epilogue(jobs[ji - 1], pos.pop(ji - 1))
                if ji >= 1 and gi == 1 and not final:
                    emit_av_kt(jobs[ji - 1], rings[ji - 1],
                               jobs[ji - 1]["kt"] - 1, pos[ji - 1])
            if ji >= 1 and not final:
                emit_epilogue(jobs[ji - 1], pos.pop(ji - 1))
                rings.pop(ji - 1)
        # Last job: its AV accumulator lives in the now-idle pool-A bank
        # region (so it never waits on the previous epilogue's po copy),
        # and its matmuls chase the exps straight down the queue.
        po_last = ps_a.tile([128, 1536], F32, tag="plA", name="po_last")
        pos[nj - 1] = po_last
        for kt in range(jobs[nj - 1]["kt"]):
            emit_av_kt(jobs[nj - 1], rings[nj - 1], kt, pos[nj - 1])
        emit_epilogue(jobs[nj - 1], pos.pop(nj - 1), last=True)

    nc.compile()
    return nc


def kernel(memory, query, seq_mask, b):
    memory = np.ascontiguousarray(memory, dtype=np.float32)
    query = np.ascontiguousarray(query, dtype=np.float32)
    seq_mask = np.asarray(seq_mask)
    assert memory.shape == (B, S, 2 * D) and query.shape == (B, S, D)

    counts = [int(np.count_nonzero(seq_mask[i])) for i in range(B)]
    tiles = [max((c + 127) // 128, 1) for c in counts]
    kt_a = max(tiles)
    big = [i for i in range(B) if tiles[i] == kt_a]
    if len(big) == B:
        big = big[:-1]
    small = [i for i in range(B) if i not in big]
    kt_b = max(tiles[i] for i in small)
    n_a = len(big)            # head-slots per core from "big" batches

    key = (kt_a, kt_b, n_a, tuple(sorted(OPTS.items())))
    if key not in _NC_CACHE:
        _NC_CACHE[key] = _build(kt_a, kt_b, n_a, OPTS)
    nc = _NC_CACHE[key]

    # Per-batch compacted/transposed operands (fp16).
    q_t = np.ascontiguousarray(query.transpose(0, 2, 1)).astype(np.float16)
    ktb_all, vab_all = {}, {}
    for i in range(B):
        kp = (kt_a if i in big else kt_b) * 128
        idx = np.flatnonzero(seq_mask[i])
        nb = len(idx)
        ktb = np.zeros((D, kp), dtype=np.float16)
        vab = np.zeros((kp, H, 129), dtype=np.float16)
        if nb:
            ktb[:, :nb] = memory[i, idx, :D].T
            vab[:nb, :, :128] = memory[i, idx, D:].reshape(nb, H, DH)
            vab[:nb, :, 128] = 1.0
        ktb_all[i], vab_all[i] = ktb, vab

    # Deal the (batch, head) pairs: core c gets n_a consecutive entries
    # of the "big" head list and 8-n_a of the "small" head list.
    heads_a = [(bi, h) for bi in big for h in range(H)]
    heads_b = [(bi, h) for bi in small for h in range(H)]
    n_b = H - n_a
    placements, in_maps = [], []
    for c in range(B):
        slots = heads_a[c * n_a:(c + 1) * n_a] + heads_b[c * n_b:(c + 1) * n_b]
        placements.append(slots)
        qrows = np.concatenate(
            [q_t[bi][h * DH:(h + 1) * DH] for bi, h in slots], axis=0)
        ka = np.concatenate(
            [ktb_all[bi][h * DH:(h + 1) * DH] for bi, h in slots[:n_a]], axis=0)
        kb = np.concatenate(
            [ktb_all[bi][h * DH:(h + 1) * DH] for bi, h in slots[n_a:]], axis=0)
        va = np.concatenate(
            [vab_all[bi][:, h] for bi, h in slots[:n_a]], axis=1)
        vb = np.concatenate(
            [vab_all[bi][:, h] for bi, h in slots[n_a:]], axis=1)
        in_maps.append({
            "q_t": np.ascontiguousarray(qrows),
            "k_a": np.ascontiguousarray(ka),
            "k_b": np.ascontiguousarray(kb),
            "v_a": np.ascontiguousarray(va),
            "v_b": np.ascontiguousarray(vb),
        })

    res = run_bass_kernel_spmd(nc, in_maps, list(range(B)))
    out = np.empty((B, S, D), dtype=np.float32)
    for c, slots in enumerate(placements):
        o = res.results[c]["out_t"].astype(np.float32).reshape(H, 128, S)
        for j, (bi, h) in enumerate(slots):
            # [p, (qi d)] -> [qi, p, d] -> [S, d]
            blk = o[j].reshape(128, NQT, DH).transpose(1, 0, 2)
            out[bi][:, h * DH:(h + 1) * DH] = blk.reshape(S, DH)
    for i in range(B):
        if counts[i] == 0:
            # all keys masked: reference softmax degenerates to uniform
            out[i] = memory[i, :, D:].mean(axis=0)[None, :]
    return out

